# revision 45
# baseline (speedup 1.0000x reference)
"""DeformConv2d (B=8, C=64, H=W=64, K=3) on 8 Trainium2 NeuronCores.

Batch-parallel: one image per core, all cores run the same Bass/Tile
program.

Math (tent formulation of bilinear sampling; offsets satisfy |dy|,|dx|<1
so each axis' bilinear weight is the 3-point tent (relu(-d), 1-|d|,
relu(d)) on the 3 integer neighbours; out-of-image taps vanish because we
sample a zero-padded image):

  out[o,p] = sum_{k,u,v} wy_u[k,p]*wx_v[k,p]*xpad[c, p+shift(k,u,v)]
             contracted with d_w[o,c,k] over (c,k).

Kernel phases (fp16 data path, fp32 PSUM accumulation):
  0. host-prepadded image pair xpad2 [128, 68*68] fp16 loads in two column
     chunks (rows 64-127 hold the image shifted down one row, so a single
     AP covers two row-adjacent terms); small DMAs load the offset-conv
     weights (first, so the conv starts early), biases, and per-pass d_w.
     Dep-free dummy matmuls on a memset scratch warm the PE p-state while
     the loads land.
  1. offset conv: 6 pair-packed K=128 fp16 matmuls per 512-pixel chunk
     into PSUM, ACT-evacuated (+p_b) to off [18, 4096] fp16 with channels
     host-permuted to dy_0..8 | dx_0..8.
  2. refold to dyf/dxf [36, 1024] (row k*4+b, pixel 1024*b+q) via 2 DMAs
     whose partition crossings are all clean multiples of the AP stride
     (the interp/lowering cannot start at an odd partition with a stride-2
     partition dim); 8 DVE ops build both axes' tent factors; 9 products
     (6 DVE + 3 Pool) build the 81 fields in wall [36, 9*1024] fp16, each
     product's column slice immediately storing out to wtab_uv [9, 4096]
     DRAM so the stores hide under the product chain.
  3. 41 passes (the 81 terms pair two-per-pass; the 5 leftover-pair
     "split" passes interleave evenly through the schedule): two regular
     stride-0-broadcast DMAs replicate one wtab row across each 64-
     partition half of wexp; the modulate splits by pixel rows across DVE
     (fast 2-byte tensor_mul, rows 0-55) and Pool (rows 56-63); 8 fp16
     matmuls accumulate d_w^T @ modulated into PSUM (contraction 128 = two
     terms at once).
  4. evac (+d_b) alternates ACT/DVE (Pool cannot read PSUM), fp16
     store in two chunks.

kernel(**inputs) takes full (unsharded) inputs, returns the full output.
"""

import sys

sys.path.insert(0, "/opt/trn_rl_repo")

import numpy as np
import concourse.bass as bass
import concourse.bacc as bacc
import concourse.mybir as mybir
from concourse.tile import TileContext
from concourse.bass_utils import run_bass_kernel_spmd

dt = mybir.dt
AF = mybir.ActivationFunctionType
OP = mybir.AluOpType

B, CIN, H, W = 8, 64, 64, 64
COUT, K = 64, 3
K2 = K * K
HP = H + 4          # 68: 2-pad each side (tent reach is rows/cols -2..65)
FP = HP * HP        # 4624
NPIX = H * W        # 4096
NCH = 8             # pixel chunks (512 each) = PSUM banks
CH = NPIX // NCH    # 512
FQ = 1024           # fold quarter (pixels per fold block)


def _make_pass_plan():
    """81 (k,u,v) terms -> 41 passes of <=2 terms.

    A pass holds terms (ta, sa) on partitions 0-63 and (tb, sb) on
    64-127.  'split' False means sb == sa+(1,0): one image AP covers both
    halves via the row-shifted copy in xpad2.  'split' True pairs two
    arbitrary-shift terms: two half-modulates (half2's base needs
    sb[0] >= -1 because the shifted copy only pads one row on top)."""
    by_shift = {}
    for k in range(K2):
        kh, kw = k // 3, k % 3
        for u in (-1, 0, 1):
            for v in (-1, 0, 1):
                by_shift.setdefault((kh - 1 + u, kw - 1 + v), []).append((k, u, v))
    passes = []
    leftovers = []
    for sx in range(-2, 3):
        col = {sy: list(by_shift.get((sy, sx), [])) for sy in range(-2, 3)}
        for sy in range(-2, 2):
            a, bb = col[sy], col[sy + 1]
            while a and bb:
                passes.append((a.pop(), (sy, sx), bb.pop(), (sy + 1, sx), False))
        for sy in range(-2, 3):
            while col[sy]:
                leftovers.append((col[sy].pop(), (sy, sx)))
    extras = []
    while len(leftovers) >= 2:
        ta, sa = leftovers.pop()
        tb, sb = leftovers.pop()
        if sb[0] < -1:
            (ta, sa), (tb, sb) = (tb, sb), (ta, sa)
        assert sb[0] >= -1
        extras.append((ta, sa, tb, sb, True))
    if leftovers:
        ta, sa = leftovers.pop()
        extras.append((ta, sa, None, None, True))
    # interleave the DVE-heavier split passes evenly through the schedule
    # instead of clustering them at the end (keeps the vector engine from
    # backlogging right before the drain)
    step = max(1, len(passes) // (len(extras) + 1))
    for i, e in enumerate(extras):
        passes.insert(min(len(passes), (i + 1) * step + i), e)
    n = sum((p[0] is not None) + (p[2] is not None) for p in passes)
    assert n == 81, n
    return passes


PASSES = _make_pass_plan()
NP_ = len(PASSES)   # 41
NCC = NP_ * 64      # fp16 consts columns: dw per pass


def _tid(term):
    k, u, v = term
    return k * 9 + (u + 1) * 3 + (v + 1)


def _pstep(ap):
    return ap.ap[0][0]


def build_nc():
    nc = bacc.Bacc(None, target_bir_lowering=False)
    f32 = dt.float32
    f16 = dt.float16

    xp_d = nc.dram_tensor("xp", [128, FP], f16, kind="ExternalInput")
    pw_d = nc.dram_tensor("pwt", [128, 108], f16, kind="ExternalInput")
    cst_d = nc.dram_tensor("cst", [128, NCC], f16, kind="ExternalInput")
    bia_d = nc.dram_tensor("bia", [128, 2], f32, kind="ExternalInput")
    y_d = nc.dram_tensor("y", [COUT, NPIX], f16, kind="ExternalOutput")
    wtabs = [nc.dram_tensor(f"wtab{uv}", [9, NPIX], f16, kind="Internal")
             for uv in range(9)]

    with TileContext(nc) as tc:
        with (
            tc.tile_pool(name="const", bufs=1) as cp,
            tc.tile_pool(name="wexp", bufs=8) as wp,
            tc.tile_pool(name="mod", bufs=5) as mp,
            tc.tile_pool(name="psout", bufs=1, space="PSUM") as pso,
        ):
            # ---------------- phase 0: loads (one DMA each) ----------------
            # conv-critical tensors (pw, bias, image) load first; the big
            # per-pass weight table and gather indices follow
            xpad2 = cp.tile([128, FP], f16)
            XSPL = 38 * HP
            nc.sync.dma_start(out=xpad2[:, 0:XSPL], in_=xp_d[:, 0:XSPL])
            pwt = cp.tile([128, 108], f16)
            nc.scalar.dma_start(out=pwt[:], in_=pw_d[:])
            bia = cp.tile([128, 2], f32)
            nc.scalar.dma_start(out=bia[:], in_=bia_d[:])
            nc.sync.dma_start(out=xpad2[:, XSPL:FP], in_=xp_d[:, XSPL:FP])
            cst = cp.tile([128, NCC], f16)
            nc.sync.dma_start(out=cst[:], in_=cst_d[:])

            pw_sb = pwt[:, :]
            dw0 = 0
            pb_sb = bia[0:2 * K2, 0:1]
            db_sb = bia[0:COUT, 1:2]

            xt = xpad2[:, :]
            xps = _pstep(xt)

            def img_view(base, npart, poff=0):
                return bass.AP(xt.tensor, xt.offset + poff * xps + base,
                               [[xps, npart], [HP, 64], [1, W]])

            # ---------------- phase 1: offset conv ----------------
            # g<3: kw=g, taps (0,kw)+(1,kw), base (i0+1)*68+kw+1
            # g>=3: kw=g-3, tap (2,kw) on half2 (zero half1 weights),
            #       base (i0+2)*68+kw+1
            off_full = cp.tile([128, NPIX], f16)
            off = off_full[0:2 * K2, :]
            psum_out = pso.tile([COUT, NPIX], f32)

            # warm the PE p-state while the input DMAs land: dep-free dummy
            # matmuls on a memset scratch keep the engine busy through the
            # load window so the offset conv runs at full clock
            scratch = cp.tile([128, CH], f16)
            nc.gpsimd.memset(scratch[:], 0.0)
            for w in range(10):
                nc.tensor.matmul(
                    psum_out[0:2 * K2, 0:CH], scratch[:, 0:2 * K2],
                    scratch[:], start=True, stop=True)
            for c in range(NCH):
                i0 = 8 * c
                pst = psum_out[0:2 * K2, CH * c:CH * (c + 1)]
                for g in range(6):
                    kw = g % 3
                    base = (i0 + (1 if g < 3 else 2)) * HP + kw + 1
                    nc.tensor.matmul(
                        pst,
                        pw_sb[:, 18 * g:18 * g + 18],
                        bass.AP(xt.tensor, xt.offset + base,
                                [[xps, 128], [HP, 8], [1, W]]),
                        start=(g == 0),
                        stop=(g == 5),
                    )
                nc.scalar.activation(off[:, CH * c:CH * (c + 1)], pst,
                                     AF.Identity, bias=pb_sb, scale=1.0)

            # keep the PE busy through phase 2 (tents run on DVE) so the
            # p-state ramp carries into the first phase-3 passes
            for w in range(24):
                nc.tensor.matmul(
                    psum_out[0:2 * K2, 0:CH], scratch[:, 0:2 * K2],
                    scratch[:], start=True, stop=True)

            # ---------------- phase 2: tent weight fields ----------------
            # off rows are host-permuted: 0-8 = dy_0..dy_8, 9-17 = dx_0..8.
            # refold to dyf/dxf [36, 1024], row k*4+b <- off[k(+9), 1024b+q].
            # Partition dims lead on both sides and every partition base is
            # a clean multiple of the AP's partition stride.
            dyf = cp.tile([36, FQ], f16)
            dxf = cp.tile([36, FQ], f16)
            ot = off[:, :]
            ops_ = _pstep(ot)
            for par, dtile in ((0, dyf), (1, dxf)):
                src = bass.AP(ot.tensor, ot.offset + par * K2 * ops_,
                              [[ops_, K2], [FQ, 4], [1, FQ]])
                nc.sync.dma_start(out=dtile[:], in_=src)

            ay = cp.tile([36, FQ], f16)
            by = cp.tile([36, FQ], f16)
            y0 = cp.tile([36, FQ], f16)
            ax = cp.tile([36, FQ], f16)
            bx = cp.tile([36, FQ], f16)
            x0 = cp.tile([36, FQ], f16)
            wy = {-1: by, 0: y0, 1: ay}
            wx = {-1: bx, 0: x0, 1: ax}

            def emit_factor(name):
                if name == "by":
                    nc.vector.tensor_scalar(by[:], dyf[:], -1.0, 0.0,
                                            OP.mult, OP.max)
                elif name == "bx":
                    nc.vector.tensor_scalar(bx[:], dxf[:], -1.0, 0.0,
                                            OP.mult, OP.max)
                elif name == "ay":
                    nc.vector.tensor_scalar_max(ay[:], dyf[:], 0.0)
                elif name == "ax":
                    nc.vector.tensor_scalar_max(ax[:], dxf[:], 0.0)
                elif name == "y0":
                    nc.vector.tensor_add(y0[:], ay[:], by[:])
                    nc.vector.tensor_scalar(y0[:], y0[:], -1.0, 1.0,
                                            OP.mult, OP.add)
                else:
                    nc.vector.tensor_add(x0[:], ax[:], bx[:])
                    nc.vector.tensor_scalar(x0[:], x0[:], -1.0, 1.0,
                                            OP.mult, OP.add)

            # wall row k*4+b, col uv*1024+q = wy_u[k,b] * wx_v[k,b];
            # each product's column slice stores out to wtab_uv [9, 4096]
            # (row k, col b*1024+q) immediately, so the store DMAs hide
            # under the remaining product ops.  uv order follows the first
            # pass that needs each field.
            wall = cp.tile([36, 9 * FQ], f16)
            wlt = wall[:, :]
            wps = _pstep(wlt)
            uv_order = []
            for (ta, sa, tb, sb, split) in PASSES:
                for tt in (ta, tb):
                    if tt is not None and _tid(tt) % 9 not in uv_order:
                        uv_order.append(_tid(tt) % 9)
            for uv in range(9):
                if uv not in uv_order:
                    uv_order.append(uv)
            # dependency-driven emission: after each tent factor, fire any
            # pending products (in need order) whose factors are ready, so
            # the first wtab stores launch ~2us earlier
            avail = set()
            done_uv = set()

            def flush_products():
                for i, uv in enumerate(uv_order):
                    u, v = uv // 3 - 1, uv % 3 - 1
                    fy = {-1: "by", 0: "y0", 1: "ay"}[u]
                    fx = {-1: "bx", 0: "x0", 1: "ax"}[v]
                    if uv in done_uv or fy not in avail or fx not in avail:
                        continue
                    done_uv.add(uv)
                    peng = nc.gpsimd if i >= 6 else nc.vector
                    peng.tensor_mul(wall[:, FQ * uv:FQ * (uv + 1)],
                                    wy[u][:], wx[v][:])
                    dst = bass.AP(wtabs[uv], 0,
                                  [[NPIX, K2], [FQ, 4], [1, FQ]])
                    srcw = bass.AP(wlt.tensor, wlt.offset + uv * FQ,
                                   [[wps, 36], [1, FQ]])
                    nc.scalar.dma_start(out=dst, in_=srcw)

            ford = ["by", "bx", "ay", "ax", "y0", "x0"]
            # compute the first-needed product's factors first
            fuv = uv_order[0]
            f1 = {-1: "by", 0: "y0", 1: "ay"}[fuv // 3 - 1]
            f2 = {-1: "bx", 0: "x0", 1: "ax"}[fuv % 3 - 1]
            for f in (f1, f2):
                ford.remove(f)
                ford.insert(0, f)
            for f in ford:
                emit_factor(f)
                avail.add(f)
                flush_products()
            assert len(done_uv) == 9

            # ---------------- phase 3: modulated accumulation ----------------
            # per-half broadcast gathers: regular DMAs with a stride-0
            # partition dim replicate one wtab row across 64 partitions;
            # halves ride different queues (sync / scalar)
            for p, (ta, sa, tb, sb, split) in enumerate(PASSES):
                nprt = 128 if tb is not None else 64
                wexp = wp.tile([128, NPIX], f16)
                gq = nc.sync if p % 2 == 0 else nc.scalar
                ka, uva = ta[0], _tid(ta) % 9
                gq.dma_start(
                    out=wexp[0:64, :],
                    in_=bass.AP(wtabs[uva], ka * NPIX,
                                [[0, 64], [1, NPIX]]))
                if tb is not None:
                    kb, uvb = tb[0], _tid(tb) % 9
                    gq.dma_start(
                        out=wexp[64:128, :],
                        in_=bass.AP(wtabs[uvb], kb * NPIX,
                                    [[0, 64], [1, NPIX]]))
                mod = mp.tile([128, NPIX], f16)

                # modulate split by pixel rows across DVE (fast 2-byte
                # tensor_mul path, rows 0-55) and Pool (tensor_mul at 0.42
                # eff, rows 56-63; TensorScalarPtr is not supported on Pool
                # by the real ISA): per-pass latency is unchanged but DVE
                # sheds 1/8 of the load
                DR = 56
                DSPL = DR * W

                def ivw(base, npart, poff, r0, r1):
                    return bass.AP(xt.tensor,
                                   xt.offset + poff * xps + base + r0 * HP,
                                   [[xps, npart], [HP, r1 - r0], [1, W]])

                def stt(lo, hi, base, poff):
                    nc.vector.tensor_mul(
                        mod[lo:hi, 0:DSPL], ivw(base, hi - lo, poff, 0, DR),
                        wexp[lo:hi, 0:DSPL])
                    nc.gpsimd.tensor_mul(
                        mod[lo:hi, DSPL:NPIX],
                        ivw(base, hi - lo, poff, DR, H),
                        wexp[lo:hi, DSPL:NPIX])

                base1 = (sa[0] + 2) * HP + sa[1] + 2
                if not split:
                    stt(0, 128, base1, 0)
                elif tb is None:
                    stt(0, 64, base1, 0)
                else:
                    # two half-modulates with independent shifts
                    base2 = (sb[0] + 1) * HP + sb[1] + 2
                    stt(0, 64, base1, 0)
                    stt(64, 128, base2, 64)
                for c in range(NCH):
                    nc.tensor.matmul(
                        psum_out[:, CH * c:CH * (c + 1)],
                        cst[0:nprt, dw0 + 64 * p:dw0 + 64 * (p + 1)],
                        mod[0:nprt, CH * c:CH * (c + 1)],
                        start=(p == 0),
                        stop=(p == NP_ - 1),
                    )

            # ---------------- phase 4: bias + store ----------------
            # per-chunk evac + store so the output DMA pipelines behind ACT
            out_sb = cp.tile([COUT, NPIX], f16)
            for c in range(NCH):
                dst = out_sb[:, CH * c:CH * (c + 1)]
                srcp = psum_out[:, CH * c:CH * (c + 1)]
                # Pool cannot read PSUM on hardware: ACT/DVE only
                if c % 2 == 0:
                    nc.scalar.activation(dst, srcp, AF.Identity,
                                         bias=db_sb, scale=1.0)
                else:
                    nc.vector.tensor_scalar(dst, srcp, db_sb, 0.0,
                                            OP.add, OP.bypass)
                if c == 3:
                    nc.sync.dma_start(out=y_d[:, 0:4 * CH],
                                      in_=out_sb[:, 0:4 * CH])
            nc.sync.dma_start(out=y_d[:, 4 * CH:NPIX],
                              in_=out_sb[:, 4 * CH:NPIX])

    nc.compile()
    return nc


_NC = None


def _get_nc():
    global _NC
    if _NC is None:
        _NC = build_nc()
    return _NC


def _prep_shared(p_w, p_b, d_w, d_b):
    # permute offset channels to [dy_0..dy_8, dx_0..dx_8] (see phase 2)
    perm = list(range(0, 18, 2)) + list(range(1, 18, 2))
    p_w = p_w[perm]
    p_b = p_b[perm]
    pwt = np.zeros((128, 108), np.float16)
    for g in range(6):
        kw = g % 3
        if g < 3:
            pwt[0:64, 18 * g:18 * g + 18] = p_w[:, :, 0, kw].T
            pwt[64:128, 18 * g:18 * g + 18] = p_w[:, :, 1, kw].T
        else:
            pwt[64:128, 18 * g:18 * g + 18] = p_w[:, :, 2, kw].T
    cst = np.zeros((128, NCC), np.float16)
    for p, (ta, sa, tb, sb, split) in enumerate(PASSES):
        c0 = 64 * p
        k = ta[0]
        cst[0:64, c0:c0 + 64] = d_w[:, :, k // 3, k % 3].T
        if tb is not None:
            k = tb[0]
            cst[64:128, c0:c0 + 64] = d_w[:, :, k // 3, k % 3].T
    bia = np.zeros((128, 2), np.float32)
    bia[0:2 * K2, 0] = p_b
    bia[0:COUT, 1] = d_b
    return pwt, cst, bia


def _prep_xpad(xb):
    """[128, FP] fp16: rows 0-63 x at (2,2); rows 64-127 x at (1,2)."""
    xp = np.zeros((128, HP, HP), np.float16)
    xp[0:64, 2:2 + H, 2:2 + W] = xb
    xp[64:128, 1:1 + H, 2:2 + W] = xb
    return xp.reshape(128, FP)


def kernel(x, p_w, p_b, d_w, d_b):
    x = np.asarray(x, np.float32)
    p_w = np.asarray(p_w, np.float32)
    p_b = np.asarray(p_b, np.float32)
    d_w = np.asarray(d_w, np.float32)
    d_b = np.asarray(d_b, np.float32)

    pwt, cst, bia = _prep_shared(p_w, p_b, d_w, d_b)
    in_maps = [{"xp": _prep_xpad(x[b]), "pwt": pwt, "cst": cst, "bia": bia}
               for b in range(B)]
    nc = _get_nc()
    res = run_bass_kernel_spmd(nc, in_maps, core_ids=list(range(B)))
    out = np.stack([res.results[b]["y"].reshape(COUT, H, W) for b in range(B)])
    return out.astype(np.float32)
